# revision 4
# baseline (speedup 1.0000x reference)
"""Data-adaptive weight-ensembling MLP (per-sample expert-merged FFN) on 8 trn2 cores.

Math (per sample b):
  c[b,:,:]  = gate(x)[b].reshape(E, L)          (2-layer relu MLP gate)
  W1[b] = bW1 + sum_e c[b,e,0] tvW1[e];  b1[b] = bb1 + sum_e c[b,e,1] tvb1[e]
  W2[b] = bW2 + sum_e c[b,e,2] tvW2[e];  b2[b] = bb2 + sum_e c[b,e,3] tvb2[e]
  out[b] = relu(x[b] @ W1[b].T + b1[b]) @ W2[b].T + b2[b]

Design (v2):
  1. delta-fold (as before): c = gb2 + delta; the gb2-weighted expert sum is
     folded into the base weights on the host, so the device only streams the
     task-vector bank against the small (~0.07) delta coefficients.
  2. HOST gate: delta = relu(x gW1.T + gb1) gW2.T is computed exactly in fp32
     on the host (input-only function).  This removes the replicated gate
     weight stream (~1 MB/core) and the gate matmuls/transposes from the
     device, and lets all per-sample stationaries (x*delta*G1 banks, biases)
     be precomputed host-side.
  3. The per-sample effective biases bias1_eff = bb1' + delta1 tvb1 and
     bias2_eff are computed on the host and injected FOR FREE through the
     partition-reduction matmul (see 5).
  4. DMA-lean streams: banks are laid out [128, ...] per-partition-contiguous
     and fetched in 1 MB chunks on the sync HWDGE queue (near peak-rate
     descriptors); small stationaries ride the scalar (ACT) HWDGE queue.
  5. cfg "ct": column-tiled PE.  The bank matmul's stationary (x*delta) is
     only B=16 wide -> a plain matmul uses 16/128 PE columns.  We run FOUR
     concurrent (128x32)-tile matmuls (tile_position=(0,32g)), expert e on
     tile g=e%4, accumulating into psum[32g:32g+16, :].  A final ones-matmul
     S^T @ evac (S[p,b]=1 iff p%16==b) sums the four group slices AND the
     bias rows (DMA'd into evac partitions 16:32) in one pass.
     cfg "dr": fallback without tiling - fp8 DoubleRow pairs into a single
     [16, OSL] accumulator, bias added via DVE.
  6. Sharding (8 cores): DFF split 8x512; core k computes its local relu
     exactly, contracts layer 2 over its f-slice, host sums the 8 partials.

Scales: bank fp8 * S1=64, stationary x*delta*G1=16 fp8, base weights bf16 *
SC=S1*G1, biases bf16 * SC; everything lands in PSUM at scale SC and is
unscaled on evacuation.
"""

import contextlib

import numpy as np

B, D, DFF, E, L = 16, 1024, 4096, 16, 4
NCORES = 8
OSL = DFF // NCORES          # 512: per-core DFF slice
KC1 = D // 128               # 8 k-chunks for the d contraction
KC2 = OSL // 128             # 4 k-chunks for the f contraction
NCH = 8                      # bank DMA chunks (2 experts each)
EPC = E // NCH               # experts per chunk = 2
S1 = 64.0                    # fp8 scale on the tv banks
G1 = 16.0                    # fp8 scale on the stationary x*delta
SC = S1 * G1                 # resulting PSUM scale

_cache = {}

CFG = "ct"


def _build(reps: int = 1, collective: bool = False, cfg: str | None = None):
    import concourse.bacc as bacc
    import concourse.bass as bass  # noqa: F401
    import concourse.tile as tile
    import concourse.mybir as mybir
    from concourse.masks import make_identity

    if cfg is None:
        cfg = CFG
    f32 = mybir.dt.float32
    bf16 = mybir.dt.bfloat16
    f8 = mybir.dt.float8e4
    mlt = mybir.AluOpType.mult
    mx = mybir.AluOpType.max
    add = mybir.AluOpType.add
    ct = cfg == "ct"
    DR = None if ct else mybir.MatmulPerfMode.DoubleRow
    PAIR = 1 if ct else 2
    nc = bacc.Bacc("TRN2", target_bir_lowering=False, debug=False,
                   num_devices=NCORES)

    # ---- I/O (per-core data supplied via in_maps) ----
    tv1_h = nc.dram_tensor("tv1", [NCH, 128, EPC * KC1 * OSL], f8,
                           kind="ExternalInput")
    bw1_h = nc.dram_tensor("bw1", [128, KC1, OSL], bf16, kind="ExternalInput")
    tv2_h = nc.dram_tensor("tv2", [NCH, 128, EPC * 2 * KC2 * 512], f8,
                           kind="ExternalInput")
    bw2_h = nc.dram_tensor("bw2", [128, KC2, D], bf16, kind="ExternalInput")
    xt_h = nc.dram_tensor("xt", [128, KC1, B], bf16, kind="ExternalInput")
    x1b_h = nc.dram_tensor("x1b", [128, E * KC1 * B], f8,
                           kind="ExternalInput")
    cbc2_h = nc.dram_tensor("cbc2", [128, E, B], bf16, kind="ExternalInput")
    s16_h = nc.dram_tensor("s16", [128, B], bf16, kind="ExternalInput")
    b1sc_h = nc.dram_tensor("b1sc", [B, OSL], bf16, kind="ExternalInput")
    b2sc_h = nc.dram_tensor("b2sc", [2, B, 512], bf16, kind="ExternalInput")
    out_h = nc.dram_tensor("out", [B, D], bf16, kind="ExternalOutput")

    with tile.TileContext(nc) as tc, contextlib.ExitStack() as ctx:
        const = ctx.enter_context(tc.tile_pool(name="const", bufs=1))
        small = ctx.enter_context(tc.tile_pool(name="small", bufs=1))
        bankp1 = ctx.enter_context(tc.tile_pool(name="bankp1", bufs=1))
        bankp2 = ctx.enter_context(tc.tile_pool(name="bankp2", bufs=1))
        pacc = ctx.enter_context(tc.tile_pool(name="pacc", bufs=1,
                                              space="PSUM"))
        pacc2 = ctx.enter_context(tc.tile_pool(name="pacc2", bufs=2,
                                               space="PSUM"))
        prr = ctx.enter_context(tc.tile_pool(name="prr", bufs=1,
                                             space="PSUM"))
        ptp = ctx.enter_context(tc.tile_pool(name="ptp", bufs=2,
                                             space="PSUM"))

        # constants (once)
        ident16 = const.tile([B, B], f32)
        make_identity(nc, ident16[:])

        NPART = 128 if ct else 32   # evac tiles: 4 group slices + bias row

        for _rep in range(reps):
            sfx = f"_{_rep}"

            # ---- bank stream on the sync HWDGE queue (big chunks) ----
            tv1t = []
            for c in range(NCH):
                t = bankp1.tile([128, EPC, KC1, OSL], f8, tag=f"tv1_{c}")
                nc.sync.dma_start(out=t[:], in_=tv1_h.ap()[c])
                tv1t.append(t)
            bw1t = small.tile([128, KC1, OSL], bf16, name="bw1t" + sfx,
                              tag="bw1t")
            nc.sync.dma_start(out=bw1t[:], in_=bw1_h.ap())
            bw2t = small.tile([128, KC2, D], bf16, name="bw2t" + sfx,
                              tag="bw2t")
            nc.sync.dma_start(out=bw2t[:], in_=bw2_h.ap())
            tv2t = []
            for c in range(NCH):
                t = bankp2.tile([128, EPC, 2, KC2, 512], f8, tag=f"tv2_{c}")
                nc.sync.dma_start(out=t[:], in_=tv2_h.ap()[c])
                tv2t.append(t)

            # ---- small stationaries on the scalar (ACT) HWDGE queue ----
            xT = small.tile([128, KC1, B], bf16, name="xT" + sfx, tag="xT")
            nc.scalar.dma_start(out=xT[:], in_=xt_h.ap())
            x1b = small.tile([128, E, KC1, B], f8, name="x1b" + sfx,
                             tag="x1b")
            nc.scalar.dma_start(out=x1b[:], in_=x1b_h.ap())
            cbc2 = small.tile([128, E, B], bf16, name="cbc2" + sfx,
                              tag="cbc2")
            nc.scalar.dma_start(out=cbc2[:], in_=cbc2_h.ap())
            s16 = small.tile([128, B], bf16, name="s16" + sfx, tag="s16")
            nc.scalar.dma_start(out=s16[:], in_=s16_h.ap())

            # evac tiles: group partials land in [32g:32g+16], bias rows in
            # [16:32]; remaining partitions stay zero (memset) so the
            # S-matmul reduction ignores them.
            evac1 = small.tile([NPART, OSL], bf16, name="evac1" + sfx,
                               tag="evac1")
            nc.vector.memset(evac1[:], 0.0)
            nc.scalar.dma_start(out=evac1[16:32, :], in_=b1sc_h.ap())
            evac2 = []
            for n in range(2):
                t = small.tile([NPART, 512], bf16, name=f"evac2_{n}" + sfx,
                               tag=f"evac2_{n}")
                nc.vector.memset(t[:], 0.0)
                nc.scalar.dma_start(out=t[16:32, :], in_=b2sc_h.ap()[n])
                evac2.append(t)

            def grp(e):
                return (e % 4) if ct else 0

            # ---- layer 1: col-tiled psum accumulation over the f-slice ----
            psum1 = pacc.tile([NPART, OSL], f32, tag="psum1")
            started = set()
            for c in range(NCH):
                for kc in range(0, KC1, PAIR):
                    for el in range(EPC):
                        e = c * EPC + el
                        g = grp(e)
                        nc.tensor.matmul(
                            psum1[32 * g:32 * g + 16, :],
                            x1b[:, e, kc:kc + PAIR, :],
                            tv1t[c][:, el, kc:kc + PAIR, :],
                            start=(g not in started), stop=False,
                            perf_mode=DR,
                            tile_position=(0, 32 * g) if ct else None)
                        started.add(g)
            # base: 2 k-chunks per group, closes each group's accumulation
            for kc in range(KC1):
                g = grp(kc % 4)
                nc.tensor.matmul(psum1[32 * g:32 * g + 16, :],
                                 xT[:, kc, :], bw1t[:, kc, :],
                                 start=False,
                                 stop=(kc >= KC1 - (4 if ct else 1)),
                                 tile_position=(0, 32 * g) if ct else None)

            # ---- group-sum + bias via S-matmul, relu, unscale ----
            h1 = small.tile([B, OSL], f32, name="h1" + sfx, tag="h1")
            ngr = 4 if ct else 1
            for g in range(ngr):
                nc.vector.tensor_copy(evac1[32 * g:32 * g + 16, :],
                                      psum1[32 * g:32 * g + 16, :])
            pr1 = prr.tile([B, OSL], f32, tag="pr")
            nc.tensor.matmul(pr1[:], s16[0:NPART, :], evac1[:], start=True,
                             stop=True)
            nc.vector.tensor_scalar(h1[:], pr1[:], 1.0 / SC, 0.0, mlt, mx)

            # ---- transpose h1 -> h1T [128, (fc, b)] ----
            h1T = small.tile([128, KC2, B], bf16, name="h1T" + sfx,
                             tag="h1T")
            for fc in range(KC2):
                pt2 = ptp.tile([128, B], f32, tag="ps")
                nc.tensor.transpose(pt2[:], h1[:, fc * 128:(fc + 1) * 128],
                                    ident16[:])
                nc.vector.tensor_copy(h1T[:, fc, :], pt2[:])

            # ---- x2bank[p, e, fc, b] = h1T * G1*delta2 (fp8 stationary) ----
            x2bank = small.tile([128, E, KC2, B], f8, name="x2b" + sfx,
                                tag="x2b")
            nc.vector.tensor_mul(
                x2bank[:],
                h1T[:, None, :, :].broadcast_to([128, E, KC2, B]),
                cbc2[:, :, None, :].broadcast_to([128, E, KC2, B]))

            # ---- layer 2, both halves, chasing the tv2 stream ----
            psum2 = [pacc2.tile([NPART, 512], f32, name=f"psum2_{n}" + sfx,
                                tag=f"psum2_{n}")
                     for n in range(2)]
            started2 = {0: set(), 1: set()}
            for c in range(NCH):
                for n in range(2):
                    for fc in range(0, KC2, PAIR):
                        for el in range(EPC):
                            e = c * EPC + el
                            g = grp(e)
                            nc.tensor.matmul(
                                psum2[n][32 * g:32 * g + 16, :],
                                x2bank[:, e, fc:fc + PAIR, :],
                                tv2t[c][:, el, n, fc:fc + PAIR, :],
                                start=(g not in started2[n]), stop=False,
                                perf_mode=DR,
                                tile_position=(0, 32 * g) if ct else None)
                            started2[n].add(g)
            for n in range(2):
                for fc in range(KC2):
                    g = grp(fc)
                    nc.tensor.matmul(
                        psum2[n][32 * g:32 * g + 16, :],
                        h1T[:, fc, :], bw2t[:, fc, n * 512:(n + 1) * 512],
                        start=False, stop=(fc >= KC2 - (4 if ct else 1)),
                        tile_position=(0, 32 * g) if ct else None)
                for g in range(ngr):
                    nc.vector.tensor_copy(evac2[n][32 * g:32 * g + 16, :],
                                          psum2[n][32 * g:32 * g + 16, :])
                pr2 = prr.tile([B, 512], f32, tag="pr")
                nc.tensor.matmul(pr2[:], s16[0:NPART, :], evac2[n][:],
                                 start=True, stop=True)
                outp = small.tile([B, 512], bf16, name=f"outp{n}" + sfx,
                                  tag=f"outp{n}")
                nc.vector.tensor_scalar_mul(outp[:], pr2[:], 1.0 / SC)
                nc.scalar.dma_start(out=out_h.ap()[:, n * 512:(n + 1) * 512],
                                    in_=outp[:])

    nc.compile()
    return nc


def _prep_inputs(x, gW1, gb1, gW2, gb2, bW1, bb1, bW2, bb2,
                 tvW1, tvb1, tvW2, tvb2, cfg: str | None = None):
    """Build the 8 per-core in_maps (host gate + delta-fold + layouts)."""
    import ml_dtypes

    bf = np.dtype(ml_dtypes.bfloat16)
    f8 = np.dtype(ml_dtypes.float8_e4m3)
    f = np.float32
    x, gW1, gb1, gW2, gb2 = [np.asarray(a, f)
                             for a in (x, gW1, gb1, gW2, gb2)]
    bW1, bb1, bW2, bb2 = [np.asarray(a, f) for a in (bW1, bb1, bW2, bb2)]
    tvW1, tvb1, tvW2, tvb2 = [np.asarray(a, f)
                              for a in (tvW1, tvb1, tvW2, tvb2)]

    # host gate (exact): delta[b, e, l]
    h = np.maximum(x @ gW1.T + gb1, 0.0)
    delta = (h @ gW2.T).reshape(B, E, L)

    # delta-fold: base' = base + sum_e gb2[e,l] * tv[e]
    gb2r = gb2.reshape(E, L)
    bW1p = bW1 + np.tensordot(gb2r[:, 0], tvW1, axes=(0, 0))
    bb1p = bb1 + gb2r[:, 1] @ tvb1
    bW2p = bW2 + np.tensordot(gb2r[:, 2], tvW2, axes=(0, 0))
    bb2p = bb2 + gb2r[:, 3] @ tvb2

    # per-sample effective biases (SC-scaled)
    b1eff = (bb1p[None, :] + delta[:, :, 1] @ tvb1) * SC     # [B, DFF]
    b2eff = (bb2p[None, :] + delta[:, :, 3] @ tvb2) * SC     # [B, D]

    # stationaries
    xT = np.ascontiguousarray(
        x.T.reshape(KC1, 128, B).transpose(1, 0, 2)).astype(bf)
    x1b = np.clip(
        x.T.reshape(KC1, 128, B).transpose(1, 0, 2)[:, None, :, :]
        * (G1 * delta[:, :, 0].T)[None, :, None, :],
        -240.0, 240.0)
    x1b = np.ascontiguousarray(x1b.reshape(128, E * KC1 * B)).astype(f8)
    cbc2 = np.ascontiguousarray(np.broadcast_to(
        (G1 * delta[:, :, 2].T)[None, :, :], (128, E, B))).astype(bf)
    s16 = np.tile(np.eye(B, dtype=f), (8, 1)).astype(bf)

    tv1s = np.clip(tvW1 * S1, -240.0, 240.0)
    tv2s = np.clip(tvW2 * S1, -240.0, 240.0)

    in_maps = []
    for k in range(NCORES):
        o0 = k * OSL
        tv1 = (tv1s[:, o0:o0 + OSL, :]
               .reshape(E, OSL, KC1, 128).transpose(3, 0, 2, 1)
               .reshape(128, NCH, EPC * KC1 * OSL).transpose(1, 0, 2))
        tv1 = np.ascontiguousarray(tv1).astype(f8)
        bw1 = np.ascontiguousarray(
            (bW1p[o0:o0 + OSL, :].T * SC)
            .reshape(KC1, 128, OSL).transpose(1, 0, 2)).astype(bf)
        tv2 = (tv2s[:, :, o0:o0 + OSL].transpose(0, 2, 1)
               .reshape(E, KC2, 128, 2, 512).transpose(2, 0, 3, 1, 4)
               .reshape(128, NCH, EPC * 2 * KC2 * 512).transpose(1, 0, 2))
        tv2 = np.ascontiguousarray(tv2).astype(f8)
        bw2 = np.ascontiguousarray(
            (bW2p[:, o0:o0 + OSL].T * SC)
            .reshape(KC2, 128, D).transpose(1, 0, 2)).astype(bf)
        b2sc = (b2eff.reshape(B, 2, 512).transpose(1, 0, 2) if k == 0
                else np.zeros((2, B, 512), f))
        in_maps.append(dict(
            tv1=tv1, bw1=bw1, tv2=tv2, bw2=bw2,
            xt=xT, x1b=x1b, cbc2=cbc2, s16=s16,
            b1sc=np.ascontiguousarray(b1eff[:, o0:o0 + OSL]).astype(bf),
            b2sc=np.ascontiguousarray(b2sc).astype(bf),
        ))
    return in_maps


def kernel(**inputs):
    from concourse.bass_utils import run_bass_kernel_spmd

    key = ("nc", CFG)
    if key not in _cache:
        _cache[key] = _build(cfg=CFG)
    nc = _cache[key]

    in_maps = _prep_inputs(**{k: np.asarray(v) for k, v in inputs.items()},
                           cfg=CFG)
    res = run_bass_kernel_spmd(nc, in_maps, core_ids=list(range(NCORES)))
    # each core holds a partial sum over its DFF slice: unshard = sum
    out = np.zeros((B, D), np.float32)
    for r in res.results:
        out += np.asarray(r["out"], np.float32)
    return out


# revision 9
# speedup vs baseline: 1.0643x; 1.0643x over previous
"""Data-adaptive weight-ensembling MLP (per-sample expert-merged FFN) on 8 trn2 cores.

Math (per sample b):
  c[b,:,:]  = gate(x)[b].reshape(E, L)          (2-layer relu MLP gate)
  W1[b] = bW1 + sum_e c[b,e,0] tvW1[e];  b1[b] = bb1 + sum_e c[b,e,1] tvb1[e]
  W2[b] = bW2 + sum_e c[b,e,2] tvW2[e];  b2[b] = bb2 + sum_e c[b,e,3] tvb2[e]
  out[b] = relu(x[b] @ W1[b].T + b1[b]) @ W2[b].T + b2[b]

Design (v3):
  1. delta-fold: c = gb2 + delta; the sample-independent gb2-weighted expert
     sum folds into the base weights on the host, so the device streams the
     task-vector bank only against the small (~0.07) delta coefficients.
  2. HOST gate: delta = relu(x gW1.T + gb1) gW2.T computed exactly in fp32 on
     the host (function of inputs only).  Removes the replicated gate-weight
     stream and all gate compute from the device; the per-sample stationaries
     (x*delta*G1 fp8 banks) and effective biases are host-precomputed.
  3. DMA: banks laid out [128, ...] per-partition-contiguous, streamed in
     1 MB chunks over the sync HWDGE queue at near roofline; small
     stationaries ride the scalar (ACT) queue.  Stream order tv1, bw1, tv2,
     bw2 matches compute order so the PE chases the stream with ~1 MB lag.
  4. cfg "ct": column-tiled PE.  The bank matmuls' stationary (x*delta) is
     only B=16 wide, so a plain matmul uses 16/128 PE columns.  We run four
     concurrent (128x32)-tile matmuls (tile_position=(0,32g)), expert e on
     tile g=e%4, accumulating into psum[32g:32g+16, :].  PSUM tiles are
     zeroed up front (start=False throughout) so every partition is valid.
  5. Evacuation: ONE full [128,512] f32->bf16 copy per psum (ACT or DVE),
     then a ones-matmul with S[p,b] = (1/SC) * [p%16==b] sums the group
     slices and unscales in one PE pass; a second eye-matmul injects the
     host-computed per-sample bias.  L1 applies relu on DVE; L2 results DMA
     out per half.  No PE mode switches (everything is (128,32) except the
     h1 transposes).
  6. Sharding (8 cores): DFF split 8x512; core k computes its local relu
     exactly, contracts layer 2 over its f-slice, host sums the partials.

Scales: bank fp8 * S1=64, stationary x*delta*G1=16 fp8, base weights bf16 *
SC=S1*G1; PSUM partials are SC-scaled, unscaled by the reduction matmul.
"""

import contextlib

import numpy as np

B, D, DFF, E, L = 16, 1024, 4096, 16, 4
NCORES = 8
OSL = DFF // NCORES          # 512: per-core DFF slice
KC1 = D // 128               # 8 k-chunks for the d contraction
KC2 = OSL // 128             # 4 k-chunks for the f contraction
NCH = 8                      # bank DMA chunks (2 experts each)
EPC = E // NCH               # experts per chunk = 2
S1 = 64.0                    # fp8 scale on the tv banks
G1 = 16.0                    # fp8 scale on the stationary x*delta
SC = S1 * G1                 # resulting PSUM scale

_cache = {}

CFG = "ct"


def _build(reps: int = 1, collective: bool = False, cfg: str | None = None):
    import concourse.bacc as bacc
    import concourse.bass as bass  # noqa: F401
    import concourse.tile as tile
    import concourse.mybir as mybir
    from concourse.masks import make_identity

    if cfg is None:
        cfg = CFG
    f32 = mybir.dt.float32
    bf16 = mybir.dt.bfloat16
    f8 = mybir.dt.float8e4
    mlt = mybir.AluOpType.mult
    mx = mybir.AluOpType.max
    Copy = mybir.ActivationFunctionType.Copy
    ct = cfg == "ct"
    DR = None if ct else mybir.MatmulPerfMode.DoubleRow
    PAIR = 1 if ct else 2
    nc = bacc.Bacc("TRN2", target_bir_lowering=False, debug=False,
                   num_devices=NCORES)

    # ---- I/O (per-core data supplied via in_maps) ----
    tv1_h = nc.dram_tensor("tv1", [NCH, 128, EPC * KC1 * OSL], f8,
                           kind="ExternalInput")
    bw1_h = nc.dram_tensor("bw1", [128, KC1, OSL], bf16, kind="ExternalInput")
    tv2_h = nc.dram_tensor("tv2", [NCH, 128, EPC * 2 * KC2 * 512], f8,
                           kind="ExternalInput")
    bw2_h = nc.dram_tensor("bw2", [128, KC2, D], bf16, kind="ExternalInput")
    xt_h = nc.dram_tensor("xt", [128, KC1, B], bf16, kind="ExternalInput")
    x1b_h = nc.dram_tensor("x1b", [128, E * KC1 * B], f8,
                           kind="ExternalInput")
    cbc2_h = nc.dram_tensor("cbc2", [128, E, B], bf16, kind="ExternalInput")
    s16_h = nc.dram_tensor("s16", [128, B], bf16, kind="ExternalInput")
    e16_h = nc.dram_tensor("e16", [128, B], bf16, kind="ExternalInput")
    b1e_h = nc.dram_tensor("b1e", [B, OSL], bf16, kind="ExternalInput")
    b2e_h = nc.dram_tensor("b2e", [B, D], bf16, kind="ExternalInput")
    out_h = nc.dram_tensor("out", [B, D], bf16, kind="ExternalOutput")

    with tile.TileContext(nc) as tc, contextlib.ExitStack() as ctx:
        const = ctx.enter_context(tc.tile_pool(name="const", bufs=1))
        small = ctx.enter_context(tc.tile_pool(name="small", bufs=1))
        bankp1 = ctx.enter_context(tc.tile_pool(name="bankp1", bufs=1))
        bankp2 = ctx.enter_context(tc.tile_pool(name="bankp2", bufs=1))
        pacc = ctx.enter_context(tc.tile_pool(name="pacc", bufs=1,
                                              space="PSUM"))
        pacc2 = ctx.enter_context(tc.tile_pool(name="pacc2", bufs=2,
                                               space="PSUM"))
        prr = ctx.enter_context(tc.tile_pool(name="prr", bufs=1,
                                             space="PSUM"))
        ptp = ctx.enter_context(tc.tile_pool(name="ptp", bufs=2,
                                             space="PSUM"))

        # constants (once)
        ident16 = const.tile([B, B], f32)
        make_identity(nc, ident16[:])

        for _rep in range(reps):
            sfx = f"_{_rep}"

            # ---- bank stream on the sync HWDGE queue (1 MB chunks) ----
            tv1t = []
            for c in range(NCH):
                t = bankp1.tile([128, EPC, KC1, OSL], f8, tag=f"tv1_{c}")
                nc.sync.dma_start(out=t[:], in_=tv1_h.ap()[c])
                tv1t.append(t)
            bw1t = small.tile([128, KC1, OSL], bf16, name="bw1t" + sfx,
                              tag="bw1t")
            nc.sync.dma_start(out=bw1t[:], in_=bw1_h.ap())
            tv2t = []
            for c in range(NCH):
                t = bankp2.tile([128, EPC, 2, KC2, 512], f8, tag=f"tv2_{c}")
                nc.sync.dma_start(out=t[:], in_=tv2_h.ap()[c])
                tv2t.append(t)
            bw2t = small.tile([128, KC2, D], bf16, name="bw2t" + sfx,
                              tag="bw2t")
            nc.sync.dma_start(out=bw2t[:], in_=bw2_h.ap())

            # ---- small stationaries on the scalar (ACT) HWDGE queue ----
            xT = small.tile([128, KC1, B], bf16, name="xT" + sfx, tag="xT")
            nc.scalar.dma_start(out=xT[:], in_=xt_h.ap())
            x1b = small.tile([128, E, KC1, B], f8, name="x1b" + sfx,
                             tag="x1b")
            nc.scalar.dma_start(out=x1b[:], in_=x1b_h.ap())
            cbc2 = small.tile([128, E, B], bf16, name="cbc2" + sfx,
                              tag="cbc2")
            nc.scalar.dma_start(out=cbc2[:], in_=cbc2_h.ap())
            s16 = small.tile([128, B], bf16, name="s16" + sfx, tag="s16")
            nc.scalar.dma_start(out=s16[:], in_=s16_h.ap())
            e16 = small.tile([128, B], bf16, name="e16" + sfx, tag="e16")
            nc.scalar.dma_start(out=e16[:], in_=e16_h.ap())
            # bias riders: [128, .] tiles, rows 0:16 = per-sample bias,
            # rest zero; injected via the e16 eye-matmul.
            b1t = small.tile([128, OSL], bf16, name="b1t" + sfx, tag="b1t")
            nc.vector.memset(b1t[:], 0.0)
            nc.scalar.dma_start(out=b1t[0:B, :], in_=b1e_h.ap())
            b2t = small.tile([128, D], bf16, name="b2t" + sfx, tag="b2t")
            nc.vector.memset(b2t[:], 0.0)
            nc.scalar.dma_start(out=b2t[0:B, :], in_=b2e_h.ap())

            def grp(e):
                return (e % 4) if ct else 0

            # ---- layer 1: col-tiled psum accumulation over the f-slice ----
            psum1 = pacc.tile([128, OSL], f32, tag="psum1")
            nc.vector.memset(psum1[:], 0.0)
            for c in range(NCH):
                for kc in range(0, KC1, PAIR):
                    for el in range(EPC):
                        e = c * EPC + el
                        g = grp(e)
                        nc.tensor.matmul(
                            psum1[32 * g:32 * g + 16, :],
                            x1b[:, e, kc:kc + PAIR, :],
                            tv1t[c][:, el, kc:kc + PAIR, :],
                            start=False, stop=False,
                            perf_mode=DR, skip_group_check=True,
                            tile_position=(0, 32 * g) if ct else None)
            # base: closes each group's accumulation
            for kc in range(KC1):
                g = grp(kc % 4)
                nc.tensor.matmul(psum1[32 * g:32 * g + 16, :],
                                 xT[:, kc, :], bw1t[:, kc, :],
                                 start=False,
                                 stop=(kc >= KC1 - (4 if ct else 1)),
                                 skip_group_check=True,
                                 tile_position=(0, 32 * g) if ct else None)

            # ---- evac + group-sum/unscale + bias + relu ----
            evac1 = small.tile([128, OSL], bf16, name="evac1" + sfx,
                               tag="evac1")
            nc.scalar.activation(evac1[:], psum1[:], Copy)
            pr1 = prr.tile([B, OSL], f32, tag="pr")
            nc.tensor.matmul(pr1[:], s16[:], evac1[:], start=True,
                             stop=False, skip_group_check=True)
            nc.tensor.matmul(pr1[:], e16[:], b1t[:], start=False,
                             stop=True, skip_group_check=True)
            h1 = small.tile([B, OSL], f32, name="h1" + sfx, tag="h1")
            nc.vector.tensor_scalar(h1[:], pr1[:], 1.0, 0.0, mlt, mx)

            # ---- transpose h1 -> h1T [128, (fc, b)] ----
            h1T = small.tile([128, KC2, B], bf16, name="h1T" + sfx,
                             tag="h1T")
            for fc in range(KC2):
                pt2 = ptp.tile([128, B], f32, tag="ps")
                nc.tensor.transpose(pt2[:], h1[:, fc * 128:(fc + 1) * 128],
                                    ident16[:])
                nc.vector.tensor_copy(h1T[:, fc, :], pt2[:])

            # ---- x2bank[p, e, fc, b] = h1T * G1*delta2 (fp8 stationary) ----
            x2bank = small.tile([128, E, KC2, B], f8, name="x2b" + sfx,
                                tag="x2b")
            nc.vector.tensor_mul(
                x2bank[:],
                h1T[:, None, :, :].broadcast_to([128, E, KC2, B]),
                cbc2[:, :, None, :].broadcast_to([128, E, KC2, B]))

            # ---- layer 2, both halves, chasing the tv2 stream ----
            psum2 = []
            for n in range(2):
                t = pacc2.tile([128, 512], f32, name=f"psum2_{n}" + sfx,
                               tag=f"psum2_{n}")
                nc.vector.memset(t[:], 0.0)
                psum2.append(t)
            for c in range(NCH):
                for n in range(2):
                    for fc in range(0, KC2, PAIR):
                        for el in range(EPC):
                            e = c * EPC + el
                            g = grp(e)
                            nc.tensor.matmul(
                                psum2[n][32 * g:32 * g + 16, :],
                                x2bank[:, e, fc:fc + PAIR, :],
                                tv2t[c][:, el, n, fc:fc + PAIR, :],
                                start=False, stop=False,
                                perf_mode=DR, skip_group_check=True,
                                tile_position=(0, 32 * g) if ct else None)
            for n in range(2):
                for fc in range(KC2):
                    g = grp(fc)
                    nc.tensor.matmul(
                        psum2[n][32 * g:32 * g + 16, :],
                        h1T[:, fc, :], bw2t[:, fc, n * 512:(n + 1) * 512],
                        start=False, stop=(fc >= KC2 - (4 if ct else 1)),
                        skip_group_check=True,
                        tile_position=(0, 32 * g) if ct else None)
            for n in range(2):
                evac2 = small.tile([128, 512], bf16, name=f"evac2_{n}" + sfx,
                                   tag=f"evac2_{n}")
                if n == 0:
                    nc.scalar.activation(evac2[:], psum2[n][:], Copy)
                else:
                    nc.vector.tensor_copy(evac2[:], psum2[n][:])
                pr2 = prr.tile([B, 512], f32, tag="pr")
                nc.tensor.matmul(pr2[:], s16[:], evac2[:], start=True,
                                 stop=False, skip_group_check=True)
                nc.tensor.matmul(pr2[:], e16[:],
                                 b2t[:, n * 512:(n + 1) * 512],
                                 start=False, stop=True,
                                 skip_group_check=True)
                outp = small.tile([B, 512], bf16, name=f"outp{n}" + sfx,
                                  tag=f"outp{n}")
                if n == 0:
                    nc.scalar.activation(outp[:], pr2[:], Copy)
                else:
                    nc.vector.tensor_copy(outp[:], pr2[:])
                nc.scalar.dma_start(out=out_h.ap()[:, n * 512:(n + 1) * 512],
                                    in_=outp[:])

    nc.compile()
    return nc


def _prep_inputs(x, gW1, gb1, gW2, gb2, bW1, bb1, bW2, bb2,
                 tvW1, tvb1, tvW2, tvb2, cfg: str | None = None):
    """Build the 8 per-core in_maps (host gate + delta-fold + layouts)."""
    import ml_dtypes

    bf = np.dtype(ml_dtypes.bfloat16)
    f8 = np.dtype(ml_dtypes.float8_e4m3)
    f = np.float32
    x, gW1, gb1, gW2, gb2 = [np.asarray(a, f)
                             for a in (x, gW1, gb1, gW2, gb2)]
    bW1, bb1, bW2, bb2 = [np.asarray(a, f) for a in (bW1, bb1, bW2, bb2)]
    tvW1, tvb1, tvW2, tvb2 = [np.asarray(a, f)
                              for a in (tvW1, tvb1, tvW2, tvb2)]

    # host gate (exact): delta[b, e, l]
    h = np.maximum(x @ gW1.T + gb1, 0.0)
    delta = (h @ gW2.T).reshape(B, E, L)

    # delta-fold: base' = base + sum_e gb2[e,l] * tv[e]
    gb2r = gb2.reshape(E, L)
    bW1p = bW1 + np.tensordot(gb2r[:, 0], tvW1, axes=(0, 0))
    bb1p = bb1 + gb2r[:, 1] @ tvb1
    bW2p = bW2 + np.tensordot(gb2r[:, 2], tvW2, axes=(0, 0))
    bb2p = bb2 + gb2r[:, 3] @ tvb2

    # per-sample effective biases (unscaled; injected via eye-matmul)
    b1eff = bb1p[None, :] + delta[:, :, 1] @ tvb1     # [B, DFF]
    b2eff = bb2p[None, :] + delta[:, :, 3] @ tvb2     # [B, D]

    # stationaries
    xT = np.ascontiguousarray(
        x.T.reshape(KC1, 128, B).transpose(1, 0, 2)).astype(bf)
    x1b = np.clip(
        x.T.reshape(KC1, 128, B).transpose(1, 0, 2)[:, None, :, :]
        * (G1 * delta[:, :, 0].T)[None, :, None, :],
        -240.0, 240.0)
    x1b = np.ascontiguousarray(x1b.reshape(128, E * KC1 * B)).astype(f8)
    cbc2 = np.ascontiguousarray(np.broadcast_to(
        (G1 * delta[:, :, 2].T)[None, :, :], (128, E, B))).astype(bf)
    # S: group-sum + unscale; E16: bias eye
    s16 = (np.tile(np.eye(B, dtype=f), (8, 1)) / SC).astype(bf)
    e16 = np.zeros((128, B), f)
    e16[:B, :] = np.eye(B, dtype=f)
    e16 = e16.astype(bf)

    tv1s = np.clip(tvW1 * S1, -240.0, 240.0)
    tv2s = np.clip(tvW2 * S1, -240.0, 240.0)

    in_maps = []
    for k in range(NCORES):
        o0 = k * OSL
        tv1 = (tv1s[:, o0:o0 + OSL, :]
               .reshape(E, OSL, KC1, 128).transpose(3, 0, 2, 1)
               .reshape(128, NCH, EPC * KC1 * OSL).transpose(1, 0, 2))
        tv1 = np.ascontiguousarray(tv1).astype(f8)
        bw1 = np.ascontiguousarray(
            (bW1p[o0:o0 + OSL, :].T * SC)
            .reshape(KC1, 128, OSL).transpose(1, 0, 2)).astype(bf)
        tv2 = (tv2s[:, :, o0:o0 + OSL].transpose(0, 2, 1)
               .reshape(E, KC2, 128, 2, 512).transpose(2, 0, 3, 1, 4)
               .reshape(128, NCH, EPC * 2 * KC2 * 512).transpose(1, 0, 2))
        tv2 = np.ascontiguousarray(tv2).astype(f8)
        bw2 = np.ascontiguousarray(
            (bW2p[:, o0:o0 + OSL].T * SC)
            .reshape(KC2, 128, D).transpose(1, 0, 2)).astype(bf)
        in_maps.append(dict(
            tv1=tv1, bw1=bw1, tv2=tv2, bw2=bw2,
            xt=xT, x1b=x1b, cbc2=cbc2, s16=s16, e16=e16,
            b1e=np.ascontiguousarray(b1eff[:, o0:o0 + OSL]).astype(bf),
            b2e=(np.ascontiguousarray(b2eff).astype(bf) if k == 0
                 else np.zeros((B, D), bf)),
        ))
    return in_maps


def kernel(**inputs):
    from concourse.bass_utils import run_bass_kernel_spmd

    key = ("nc", CFG)
    if key not in _cache:
        _cache[key] = _build(cfg=CFG)
    nc = _cache[key]

    in_maps = _prep_inputs(**{k: np.asarray(v) for k, v in inputs.items()},
                           cfg=CFG)
    res = run_bass_kernel_spmd(nc, in_maps, core_ids=list(range(NCORES)))
    # each core holds a partial sum over its DFF slice: unshard = sum
    out = np.zeros((B, D), np.float32)
    for r in res.results:
        out += np.asarray(r["out"], np.float32)
    return out
